# revision 3
# baseline (speedup 1.0000x reference)
"""Trainium2 Bass kernel for CrossMultiAttentionSNPPheno.

Per-core sharding: 8 cores over (batch=2) x (head-pairs=4).  Core c handles
batch b=c//4 and heads h0=2*(c%4), h0+1.  Each core computes its two heads'
columns of both outputs; host reassembles.

Math per head (no-max softmax is exact here: |scores| <= ~1.5):
  E = exp(k q^T / 8);  out_pheno = diag(1/rowsum) E vs + v_pheno
                       out_snp   = diag(1/colsum) E^T vp + v_snp
Scores are computed twice (s-major and p-major orientations) because the PE
contracts over the partition dim only; exp's accum_out gives the row/col sums
for free.  Big matmuls run as float32r (1 cyc/row at N=512).
"""

import sys
from contextlib import ExitStack

import numpy as np

sys.path.insert(0, "/opt/trn_rl_repo")

SP, SS, D, DH, H = 2048, 4096, 512, 64, 8
D2 = 2 * DH  # two heads per core

_NC_CACHE = {}


def build_nc(sp=SP, ss=SS):
    import concourse.bass as bass  # noqa: F401
    import concourse.tile as tile
    from concourse import bacc, mybir

    key = (sp, ss)
    if key in _NC_CACHE:
        return _NC_CACHE[key]

    f32 = mybir.dt.float32
    f32r = mybir.dt.float32r
    Exp = mybir.ActivationFunctionType.Exp
    Copy = mybir.ActivationFunctionType.Copy
    AX = mybir.AxisListType.X
    ADD = mybir.AluOpType.add

    NPB, NSB = sp // 128, ss // 128       # 128-row blocks
    ASUP = min(1024, sp)                  # A-pass p-super size
    NASUP, APG = sp // ASUP, ASUP // 512
    BSUP = min(1024, ss)                  # B-pass s-super size
    NBSUP, BSC = ss // BSUP, BSUP // 512

    nc = bacc.Bacc("TRN2", target_bir_lowering=False, debug=False)
    ph_d = nc.dram_tensor("ph", [sp, D], f32, kind="ExternalInput").ap()
    sn_d = nc.dram_tensor("sn", [ss, D], f32, kind="ExternalInput").ap()
    wk_d = nc.dram_tensor("wk", [D, D2], f32, kind="ExternalInput").ap()
    wq_d = nc.dram_tensor("wq", [D, D2], f32, kind="ExternalInput").ap()
    wvp_d = nc.dram_tensor("wvp", [D, D2], f32, kind="ExternalInput").ap()
    wvs_d = nc.dram_tensor("wvs", [D, D2], f32, kind="ExternalInput").ap()
    id_d = nc.dram_tensor("ident", [128, 128], f32, kind="ExternalInput").ap()
    op_d = nc.dram_tensor("op", [sp, D2], f32, kind="ExternalOutput").ap()
    os_d = nc.dram_tensor("os", [ss, D2], f32, kind="ExternalOutput").ap()

    with tile.TileContext(nc) as tc, ExitStack() as ctx:
        pers = ctx.enter_context(tc.tile_pool(name="pers", bufs=1))
        kT = pers.tile([128, sp], f32r, tag="kT")    # [d2, p]
        qT = pers.tile([128, ss], f32r, tag="qT")    # [d2, s]
        vs = pers.tile([128, NSB * 128], f32r, tag="vs")  # [s%128, sb*128+d2]
        vp = pers.tile([128, NPB * 128], f32r, tag="vp")  # [p%128, pb*128+d2]
        stA = pers.tile([128, sp], f32, tag="stA")  # unscaled out_pheno^T, h0 rows 0:64
        stB = pers.tile([128, ss], f32, tag="stB")  # unscaled out_snp^T
        colp = pers.tile([128, 2 * NSB * NASUP], f32, tag="colp")
        rowp = pers.tile([128, 2 * NPB * NBSUP], f32, tag="rowp")
        csum = pers.tile([128, 2 * NSB], f32, tag="csum")
        rsum = pers.tile([128, 2 * NPB], f32, tag="rsum")
        rcol = pers.tile([128, 2 * NSB], f32, tag="rcol")
        rrow = pers.tile([128, 2 * NPB], f32, tag="rrow")
        ident = pers.tile([128, 128], f32, tag="ident")
        nc.sync.dma_start(out=ident, in_=id_d)

        # ---- transpose inputs + projections (sn first, then ph, to cap SBUF) ----
        def proj_phase(x_d, S, w_big, w_small, outT, outV, nblk):
            # x_d [S, 512] -> xT [c, p] chunks; outT [128,S] = w_big^T x^T ;
            # outV [s%128, blk*128+d2] = x w_small (per 128-block)
            with tc.tile_pool(name="ld", bufs=4) as ldp, \
                 tc.tile_pool(name="xt", bufs=1) as xtp, \
                 tc.tile_pool(name="wp", bufs=1) as wpp, \
                 tc.tile_pool(name="ps12", bufs=2, space="PSUM") as ps12:
                xT = xtp.tile([128, 4 * S], f32r, tag="xT")  # [c%128, cb*S+row]
                wb = wpp.tile([128, 4, 128], f32r, tag="wb")
                wsm = wpp.tile([128, 4, 128], f32r, tag="wsm")
                wtmp = wpp.tile([128, 8, 128], f32, tag="wtmp")
                nc.sync.dma_start(out=wtmp[:, 0:4, :], in_=w_big.rearrange("(n p) d -> p n d", p=128))
                nc.sync.dma_start(out=wtmp[:, 4:8, :], in_=w_small.rearrange("(n p) d -> p n d", p=128))
                nc.vector.tensor_copy(wb, wtmp[:, 0:4, :])
                nc.vector.tensor_copy(wsm, wtmp[:, 4:8, :])
                for pg in range(S // 512):
                    xt = ldp.tile([128, 4, 512], f32, tag="xload")
                    nc.sync.dma_start(
                        out=xt,
                        in_=x_d[pg * 512:(pg + 1) * 512, :].rearrange(
                            "(n p) c -> p n c", p=128),
                    )
                    for cb in range(4):
                        pst = ps12.tile([128, 512], f32, tag="pst")
                        for j in range(4):
                            nc.tensor.transpose(
                                pst[:, j * 128:(j + 1) * 128],
                                xt[:, j, cb * 128:(cb + 1) * 128], ident)
                        nc.vector.tensor_copy(
                            xT[:, cb * S + pg * 512: cb * S + (pg + 1) * 512], pst)
                # big projection (transposed layout, fp32r N=512)
                for pg in range(S // 512):
                    psm = ps12.tile([128, 512], f32, tag="psproj")
                    for cb in range(4):
                        nc.tensor.matmul(
                            psm,
                            lhsT=wb[:, cb, :],
                            rhs=xT[:, cb * S + pg * 512: cb * S + (pg + 1) * 512
                                   ],
                            start=(cb == 0), stop=(cb == 3))
                    nc.vector.tensor_copy(outT[:, pg * 512:(pg + 1) * 512], psm)
                # small projection (natural layout, N=128)
                for blk in range(nblk):
                    psv = ps12.tile([128, 128], f32, tag="psv")
                    for cb in range(4):
                        nc.tensor.matmul(
                            psv,
                            lhsT=xT[:, cb * S + blk * 128: cb * S + (blk + 1) * 128],
                            rhs=wsm[:, cb, :],
                            start=(cb == 0), stop=(cb == 3))
                    nc.vector.tensor_copy(outV[:, blk * 128:(blk + 1) * 128], psv)

        proj_phase(sn_d, ss, wq_d, wvs_d, qT, vs, NSB)
        proj_phase(ph_d, sp, wk_d, wvp_d, kT, vp, NPB)

        # ---- A-pass: E^T (s-major) -> unscaled out_pheno^T + colsum partials ----
        with tc.tile_pool(name="etA", bufs=3) as etp, \
             tc.tile_pool(name="psSA", bufs=2, space="PSUM") as psS, \
             tc.tile_pool(name="psaccA", bufs=1, space="PSUM") as psacc:
            for sup in range(NASUP):
                acc = {(h, pg): psacc.tile([64, 512], f32, tag=f"accA{h}_{pg}", name=f"accA{h}_{pg}")
                       for h in range(2) for pg in range(APG)}
                for sb in range(NSB):
                    for h in range(2):
                        hr = slice(h * 64, h * 64 + 64)
                        Sps = psS.tile([128, ASUP], f32, tag="SA")
                        for pg in range(APG):
                            nc.tensor.matmul(
                                Sps[:, pg * 512:(pg + 1) * 512],
                                lhsT=qT[hr, sb * 128:(sb + 1) * 128],
                                rhs=kT[hr, sup * ASUP + pg * 512:
                                       sup * ASUP + (pg + 1) * 512],
                                start=True, stop=True)
                        ET = etp.tile([128, ASUP], f32r, tag="ET")
                        ci = (h * NSB + sb) * NASUP + sup
                        nc.scalar.activation(
                            out=ET, in_=Sps, func=Exp, scale=0.125,
                            accum_out=colp[:, ci:ci + 1])
                        for pg in range(APG):
                            nc.tensor.matmul(
                                acc[(h, pg)],
                                lhsT=vs[:, sb * 128 + h * 64: sb * 128 + h * 64 + 64
                                        ],
                                rhs=ET[:, pg * 512:(pg + 1) * 512],
                                start=(sb == 0), stop=(sb == NSB - 1))
                for h in range(2):
                    for pg in range(APG):
                        nc.vector.tensor_copy(
                            stA[h * 64:h * 64 + 64,
                                sup * ASUP + pg * 512: sup * ASUP + (pg + 1) * 512],
                            acc[(h, pg)])

        # ---- B-pass: E (p-major) -> unscaled out_snp^T + rowsum partials ----
        with tc.tile_pool(name="etB", bufs=3) as etp, \
             tc.tile_pool(name="psSB", bufs=2, space="PSUM") as psS, \
             tc.tile_pool(name="psaccB", bufs=1, space="PSUM") as psacc:
            for sup in range(NBSUP):
                acc = {(h, sc): psacc.tile([64, 512], f32, tag=f"accB{h}_{sc}", name=f"accB{h}_{sc}")
                       for h in range(2) for sc in range(BSC)}
                for pb in range(NPB):
                    for h in range(2):
                        hr = slice(h * 64, h * 64 + 64)
                        Sps = psS.tile([128, BSUP], f32, tag="SB")
                        for sc in range(BSC):
                            nc.tensor.matmul(
                                Sps[:, sc * 512:(sc + 1) * 512],
                                lhsT=kT[hr, pb * 128:(pb + 1) * 128],
                                rhs=qT[hr, sup * BSUP + sc * 512:
                                       sup * BSUP + (sc + 1) * 512],
                                start=True, stop=True)
                        E = etp.tile([128, BSUP], f32r, tag="E")
                        ri = (h * NPB + pb) * NBSUP + sup
                        nc.scalar.activation(
                            out=E, in_=Sps, func=Exp, scale=0.125,
                            accum_out=rowp[:, ri:ri + 1])
                        for sc in range(BSC):
                            nc.tensor.matmul(
                                acc[(h, sc)],
                                lhsT=vp[:, pb * 128 + h * 64: pb * 128 + h * 64 + 64
                                        ],
                                rhs=E[:, sc * 512:(sc + 1) * 512],
                                start=(pb == 0), stop=(pb == NPB - 1))
                for h in range(2):
                    for sc in range(BSC):
                        nc.vector.tensor_copy(
                            stB[h * 64:h * 64 + 64,
                                sup * BSUP + sc * 512: sup * BSUP + (sc + 1) * 512],
                            acc[(h, sc)])

        # ---- finalize sums ----
        for h in range(2):
            nc.vector.tensor_reduce(
                out=csum[:, h * NSB:(h + 1) * NSB],
                in_=colp[:, h * NSB * NASUP:(h + 1) * NSB * NASUP].rearrange(
                    "p (a b) -> p a b", b=NASUP),
                axis=AX, op=ADD)
            nc.vector.tensor_reduce(
                out=rsum[:, h * NPB:(h + 1) * NPB],
                in_=rowp[:, h * NPB * NBSUP:(h + 1) * NPB * NBSUP].rearrange(
                    "p (a b) -> p a b", b=NBSUP),
                axis=AX, op=ADD)
        nc.vector.reciprocal(rcol, csum)
        nc.vector.reciprocal(rrow, rsum)

        # ---- epilogue: transpose, scale, residual, store ----
        with tc.tile_pool(name="eps", bufs=4) as eps, \
             tc.tile_pool(name="pse", bufs=4, space="PSUM") as pse:
            for pb in range(NPB):
                stg = eps.tile([128, 128], f32, tag="stg")
                for h in range(2):
                    hr = slice(h * 64, h * 64 + 64)
                    tp = pse.tile([128, 64], f32, tag="tp")
                    nc.tensor.transpose(
                        tp, stA[hr, pb * 128:(pb + 1) * 128], ident[hr, hr])
                    nc.scalar.activation(
                        out=stg[:, h * 64:h * 64 + 64], in_=tp, func=Copy,
                        scale=rrow[:, h * NPB + pb: h * NPB + pb + 1])
                    nc.vector.tensor_add(
                        stg[:, h * 64:h * 64 + 64], stg[:, h * 64:h * 64 + 64],
                        vp[:, pb * 128 + h * 64: pb * 128 + h * 64 + 64].bitcast(f32))
                nc.sync.dma_start(out=op_d[pb * 128:(pb + 1) * 128, :], in_=stg)
            for sb in range(NSB):
                stg = eps.tile([128, 128], f32, tag="stg")
                for h in range(2):
                    hr = slice(h * 64, h * 64 + 64)
                    tp = pse.tile([128, 64], f32, tag="tp")
                    nc.tensor.transpose(
                        tp, stB[hr, sb * 128:(sb + 1) * 128], ident[hr, hr])
                    nc.scalar.activation(
                        out=stg[:, h * 64:h * 64 + 64], in_=tp, func=Copy,
                        scale=rcol[:, h * NSB + sb: h * NSB + sb + 1])
                    nc.vector.tensor_add(
                        stg[:, h * 64:h * 64 + 64], stg[:, h * 64:h * 64 + 64],
                        vs[:, sb * 128 + h * 64: sb * 128 + h * 64 + 64].bitcast(f32))
                nc.sync.dma_start(out=os_d[sb * 128:(sb + 1) * 128, :], in_=stg)

    nc.compile()
    _NC_CACHE[key] = nc
    return nc


def make_in_maps(pheno, snps, Wq, Wk, Wv_snp, Wv_pheno):
    ident = np.eye(128, dtype=np.float32)
    maps = []
    for c in range(8):
        b, h0 = c // 4, 2 * (c % 4)
        cs = slice(h0 * DH, h0 * DH + D2)
        maps.append({
            "ph": np.ascontiguousarray(pheno[b]),
            "sn": np.ascontiguousarray(snps[b]),
            "wk": np.ascontiguousarray(Wk[:, cs]),
            "wq": np.ascontiguousarray(Wq[:, cs]),
            "wvp": np.ascontiguousarray(Wv_pheno[:, cs]),
            "wvs": np.ascontiguousarray(Wv_snp[:, cs]),
            "ident": ident,
        })
    return maps


def kernel(pheno_encoded, SNPS_encoded, Wq, Wk, Wv_snp, Wv_pheno):
    from concourse.bass_utils import run_bass_kernel_spmd

    nc = build_nc()
    in_maps = make_in_maps(np.asarray(pheno_encoded, np.float32),
                           np.asarray(SNPS_encoded, np.float32),
                           np.asarray(Wq, np.float32), np.asarray(Wk, np.float32),
                           np.asarray(Wv_snp, np.float32),
                           np.asarray(Wv_pheno, np.float32))
    res = run_bass_kernel_spmd(nc, in_maps, core_ids=list(range(8)))
    B = pheno_encoded.shape[0]
    out_pheno = np.empty((B, SP, D), np.float32)
    out_snp = np.empty((B, SS, D), np.float32)
    for c in range(8):
        b, h0 = c // 4, 2 * (c % 4)
        cs = slice(h0 * DH, h0 * DH + D2)
        out_pheno[b, :, cs] = res.results[c]["op"]
        out_snp[b, :, cs] = res.results[c]["os"]
    return out_pheno, out_snp
